# revision 23
# baseline (speedup 1.0000x reference)
"""Trainium2 Bass kernel for nn_F1Layer (gnn_message_passing).

Row-shard n=4096 across 8 NeuronCores (512 rows/core). Four SPMD launches
(3 distinct NEFFs), zero device collectives — cross-core exchanges ride host
gathers between launches (cheaper than the ~70-100us ncfw collective cold
cost per NEFF):

  L1 (NEFF-A): H1_rows = A_rows @ H          (host gathers H1_full)
  L2 (NEFF-A): H2_rows = A_rows @ H1_full    (host gathers H2_full)
  host:        Yk = Hk@Uk, Gram, M_inv, Qf/QfT, scaled laplacian source —
               all tiny (O(n r) / O(r^3))
  L3 (NEFF-B): per-hop subspace attention with scores computed TRANSPOSED so
               the softmax contraction lands on the PE partition axis, masked
               exp, row-normalized aggregation, laplacian term, soft-threshold
               + residual + layernorm -> H_out rows (host gathers H_out)
  L4 (NEFF-C): lap_smooth partials = rowsum(H_out_r * (L_r @ H_out_full))

All matmuls use natural (host-pre-transposed) layouts; no device transposes.
The three hops' K=32 matmuls are packed at partition bases 0/32/64 so they
run concurrently in distinct PE row-groups.
"""

import os
import ml_dtypes
import numpy as np
from contextlib import ExitStack

import concourse.bass as bass
import concourse.tile as tile
from concourse import bacc, mybir
from concourse.bass_utils import run_bass_kernel_spmd

N = 4096
D = 256
R = 32
K_HOPS = 3
NC = 8
RPC = N // NC          # rows per core = 512
KT = N // 128          # 32 contraction tiles
MTI = RPC // 128       # 4 row tiles per core
ETA = np.float32(0.5)
EPS_SUB = 0.5
LN_EPS = np.float32(1e-5)

F32 = mybir.dt.float32
F32R = mybir.dt.float32r
BF16 = mybir.dt.bfloat16
AX = mybir.AxisListType
ALU = mybir.AluOpType
ACTF = mybir.ActivationFunctionType

# matmul input dtype knob: "f32r" (fast fp32 path) or "f32"
MM_MODE = os.environ.get("KERNEL_MM_MODE", "f32r")
MDT = F32R if MM_MODE == "f32r" else F32

TRACE = False            # set by test harness only
LAST_EXEC_NS = []        # per-launch (label, exec_ns) when TRACE

_cache = {}


def _mm(ap):
    return ap


def _run(nc, in_maps, label):
    res = run_bass_kernel_spmd(nc, in_maps, core_ids=list(range(NC)), trace=TRACE)
    if TRACE:
        LAST_EXEC_NS.append((label, res.exec_time_ns))
    return res.results


# ---------------------------------------------------------------- NEFF-A: hop
def _build_hop():
    """y[512,256] = (at)^T[512,4096] @ x[4096,256] per core."""
    nc = bacc.Bacc("TRN2", target_bir_lowering=False, debug=False, num_devices=NC)
    at = nc.dram_tensor("at", [N, RPC], BF16, kind="ExternalInput").ap()
    x = nc.dram_tensor("x", [N, D], BF16, kind="ExternalInput").ap()
    y = nc.dram_tensor("y", [RPC, D], F32, kind="ExternalOutput").ap()

    with tile.TileContext(nc) as tc, ExitStack() as ctx:
        res = ctx.enter_context(tc.tile_pool(name="res", bufs=1))
        stream = ctx.enter_context(tc.tile_pool(name="stream", bufs=4))
        outp = ctx.enter_context(tc.tile_pool(name="outp", bufs=4))
        psum = ctx.enter_context(tc.tile_pool(name="psum", bufs=1, space="PSUM"))

        xsb = res.tile([128, KT, D], BF16)
        x_re = x.rearrange("(t p) c -> p t c", p=128)
        for c in range(4):
            nc.sync.dma_start(xsb[:, 8 * c:8 * (c + 1), :], x_re[:, 8 * c:8 * (c + 1), :])

        ps = [psum.tile([128, D], F32, tag=f"ps{m}", name=f"ps{m}") for m in range(MTI)]
        atts = {}
        for k in range(KT):
            att = stream.tile([128, RPC], BF16, tag="att", bufs=10, name=f"att{k}")
            nc.sync.dma_start(att[:], at[k * 128:(k + 1) * 128, :])
            atts[k] = att
            # drain matmuls in 8-kt dense bursts so the PE crosses the HAM
            # warm window instead of idling between per-kt DMA waits
            if k % 8 == 7:
                for kk in range(k - 7, k + 1):
                    for m in range(MTI):
                        nc.tensor.matmul(
                            ps[m][:],
                            lhsT=_mm(atts[kk][:, m * 128:(m + 1) * 128]),
                            rhs=_mm(xsb[:, kk, :]),
                            start=(kk == 0),
                            stop=(kk == KT - 1),
                        )
        for m in range(MTI):
            ot = outp.tile([128, D], F32)
            nc.vector.tensor_copy(ot[:], ps[m][:])
            nc.sync.dma_start(y[m * 128:(m + 1) * 128, :], ot[:])
    nc.compile()
    return nc


# ----------------------------------------------------------- NEFF-B: attention
def _build_main():
    nc = bacc.Bacc("TRN2", target_bir_lowering=False, debug=False, num_devices=NC)
    maskt = nc.dram_tensor("maskt", [N, RPC], BF16, kind="ExternalInput").ap()
    lt = nc.dram_tensor("lt", [N, RPC], BF16, kind="ExternalInput").ap()
    ssc = nc.dram_tensor("ssc", [N, D], BF16, kind="ExternalInput").ap()  # -eta*sum(lap_k*Hk)
    hr = nc.dram_tensor("hr", [RPC, D], F32, kind="ExternalInput").ap()
    qft3 = nc.dram_tensor("qft3", [96, N], BF16, kind="ExternalInput").ap()   # hop k at parts 32k
    yrt3 = nc.dram_tensor("yrt3", [96, RPC], BF16, kind="ExternalInput").ap()
    qf1s = [nc.dram_tensor(f"qf1_{k}", [N, 34], BF16, kind="ExternalInput").ap()
            for k in range(K_HOPS)]
    ukts3 = nc.dram_tensor("ukts3", [96, D], MDT, kind="ExternalInput").ap()  # eta*w_k*Uk^T
    thrb = nc.dram_tensor("thrb", [128, D], F32, kind="ExternalInput").ap()
    nthrb = nc.dram_tensor("nthrb", [128, D], F32, kind="ExternalInput").ap()
    gamb = nc.dram_tensor("gamb", [128, D], F32, kind="ExternalInput").ap()
    betb = nc.dram_tensor("betb", [128, D], F32, kind="ExternalInput").ap()
    hout = nc.dram_tensor("hout", [RPC, D], F32, kind="ExternalOutput").ap()

    with tile.TileContext(nc) as tc, ExitStack() as ctx:
        res = ctx.enter_context(tc.tile_pool(name="res", bufs=1))
        stream = ctx.enter_context(tc.tile_pool(name="stream", bufs=4))
        tbuf = ctx.enter_context(tc.tile_pool(name="tbuf", bufs=4))
        small = ctx.enter_context(tc.tile_pool(name="small", bufs=1))
        outp = ctx.enter_context(tc.tile_pool(name="outp", bufs=4))
        psum_s = ctx.enter_context(tc.tile_pool(name="psum_s", bufs=1, space="PSUM"))
        psum_u = ctx.enter_context(tc.tile_pool(name="psum_u", bufs=1, space="PSUM"))

        # ---- residents needed by C1 first, so its matmuls start immediately
        ssc_sb = res.tile([128, KT, D], BF16)
        ssc_re = ssc.rearrange("(t p) c -> p t c", p=128)
        for c in range(8):
            nc.sync.dma_start(ssc_sb[:, 4 * c:4 * (c + 1), :], ssc_re[:, 4 * c:4 * (c + 1), :])
        hr_sb = res.tile([128, MTI, D], F32)
        nc.sync.dma_start(hr_sb[:], hr.rearrange("(t p) c -> p t c", p=128))

        # ---- C1: x0 = -eta * L @ (sum lap_k Hk) + Hr  per i-block
        x0 = [res.tile([128, D], F32, tag=f"x0_{m}", name=f"x0_{m}") for m in range(MTI)]
        acc1 = [psum_s.tile([128, D], F32, tag="sp", bufs=5, name=f"acc1_{m}")
                for m in range(MTI)]
        for kt in range(KT):
            ltt = stream.tile([128, RPC], BF16, tag="lt")
            nc.sync.dma_start(ltt[:], lt[kt * 128:(kt + 1) * 128, :])
            for m in range(MTI):
                nc.tensor.matmul(
                    acc1[m][:],
                    lhsT=_mm(ltt[:, m * 128:(m + 1) * 128]),
                    rhs=_mm(ssc_sb[:, kt, :]),
                    start=(kt == 0),
                    stop=(kt == KT - 1),
                )
        for m in range(MTI):
            nc.vector.tensor_add(x0[m][:], acc1[m][:], hr_sb[:, m, :])

        # ---- attention residents (needed ~25us in, after C1 is streaming)
        qft_sb = res.tile([96, N], BF16)
        nc.sync.dma_start(qft_sb[:], qft3[:])
        yrt_sb = res.tile([96, RPC], BF16)
        nc.sync.dma_start(yrt_sb[:], yrt3[:])
        ukts_sb = res.tile([96, D], MDT)
        nc.sync.dma_start(ukts_sb[:], ukts3[:])
        qf1_sb = [res.tile([128, KT, 34], BF16, tag=f"qf1_{k}", name=f"qf1sb{k}") for k in range(K_HOPS)]
        for k in range(K_HOPS):
            nc.sync.dma_start(qf1_sb[k][:], qf1s[k].rearrange("(t p) c -> p t c", p=128))
        thr_sb = res.tile([128, D], F32)
        nc.sync.dma_start(thr_sb[:], thrb[:])
        nthr_sb = res.tile([128, D], F32)
        nc.sync.dma_start(nthr_sb[:], nthrb[:])
        gam_sb = res.tile([128, D], F32)
        nc.sync.dma_start(gam_sb[:], gamb[:])
        bet_sb = res.tile([128, D], F32)
        nc.sync.dma_start(bet_sb[:], betb[:])
        eps_sb = res.tile([128, 1], F32)
        nc.vector.memset(eps_sb[:], float(LN_EPS))

        # ---- A: per j-tile, all hops: scores^T -> exp -> mask -> u accumulation
        # (u-matmuls software-pipelined one j-tile behind the scores)
        ups = [psum_u.tile([34, RPC], F32, tag=f"u{k}", name=f"ups{k}") for k in range(K_HOPS)]
        pend = []          # (jt, [tm0, tm1, tm2]) not yet fed to the u-matmuls

        def drain_u(last):
            for pjt, tms in pend:
                for k in range(K_HOPS):
                    nc.tensor.matmul(
                        ups[k][:],
                        lhsT=_mm(qf1_sb[k][:, pjt, :]),
                        rhs=_mm(tms[k][:]),
                        start=(pjt == 0),
                        stop=(last and pjt == KT - 1),
                    )
            pend.clear()

        for jt in range(KT):
            mkt = stream.tile([128, RPC], BF16, tag="mask")
            nc.sync.dma_start(mkt[:], maskt[jt * 128:(jt + 1) * 128, :])
            cur_tm = []
            for k in range(K_HOPS):
                sp = psum_s.tile([128, RPC], F32, tag="sp", bufs=5)
                nc.tensor.matmul(
                    sp[:],
                    lhsT=_mm(qft_sb[32 * k:32 * k + 32, jt * 128:(jt + 1) * 128]),
                    rhs=_mm(yrt_sb[32 * k:32 * k + 32, :]),
                    start=True,
                    stop=True,
                )
                tt = tbuf.tile([128, RPC], BF16, tag="tt", bufs=8)
                nc.scalar.activation(tt[:], sp[:], ACTF.Exp)
                tm = tbuf.tile([128, RPC], BF16, tag="tm", bufs=16)
                nc.vector.tensor_mul(tm[:], tt[:], mkt[:])
                cur_tm.append(tm)
            pend.append((jt, cur_tm))
            # every 4 j-tiles, drain the pending u-matmuls as one dense
            # 12-matmul PE burst (long enough to cross the HAM warm window)
            if len(pend) == 4 and jt != KT - 1:
                drain_u(last=False)
        drain_u(last=True)

        # ---- B: vT = (u / rowsum), hop k at partitions 32k (one batched recip)
        vt_sb = small.tile([96, RPC], MDT)
        for k in range(K_HOPS):
            srow = small.tile([1, RPC], F32, tag=f"srow{k}", name=f"srow{k}")
            nc.vector.tensor_copy(srow[:], ups[k][32:33, :])
            rec = small.tile([1, RPC], F32, tag=f"rec{k}", name=f"rec{k}")
            nc.vector.reciprocal_approx_fast(rec[:], srow[:])
            bc = small.tile([32, RPC], F32, tag=f"bc{k}", name=f"bc{k}")
            nc.gpsimd.partition_broadcast(bc[:], rec[:])
            nc.vector.tensor_mul(vt_sb[32 * k:32 * k + 32, :], ups[k][0:32, :], bc[:])

        # ---- C2: g accumulation + epilogue per i-block
        acc2 = [psum_s.tile([128, D], F32, tag="sp", bufs=5, name=f"acc2_{m}")
                for m in range(MTI)]
        for m in range(MTI):
            for k in range(K_HOPS):
                nc.tensor.matmul(
                    acc2[m][:],
                    lhsT=_mm(vt_sb[32 * k:32 * k + 32, m * 128:(m + 1) * 128]),
                    rhs=_mm(ukts_sb[32 * k:32 * k + 32, :]),
                    start=(k == 0),
                    stop=(k == K_HOPS - 1),
                )
        sd_all = small.tile([128, MTI], F32)
        rstd_all = small.tile([128, MTI], F32)
        xcs = []
        for m in range(MTI):
            x = outp.tile([128, D], F32, tag="x")
            nc.vector.tensor_add(x[:], acc2[m][:], x0[m][:])          # H_half
            cl = outp.tile([128, D], F32, tag="cl")
            nc.vector.tensor_max(cl[:], x[:], nthr_sb[:])
            nc.vector.tensor_tensor(cl[:], cl[:], thr_sb[:], ALU.min)
            y2 = outp.tile([128, D], F32, tag="y2")
            nc.vector.tensor_sub(y2[:], x[:], cl[:])             # soft-threshold
            nc.vector.tensor_add(y2[:], y2[:], hr_sb[:, m, :])   # residual
            # layernorm stats
            red = outp.tile([128, 1], F32, tag="red")
            nc.vector.tensor_reduce(red[:], y2[:], AX.X, ALU.add)
            mu = outp.tile([128, 1], F32, tag="mu")
            nc.scalar.mul(mu[:], red[:], 1.0 / D)
            xc = outp.tile([128, D], F32, tag=f"xc{m}", name=f"xc{m}")
            nc.vector.tensor_scalar_sub(xc[:], y2[:], mu[:])
            var = outp.tile([128, 1], F32, tag="var")
            sq = outp.tile([128, D], F32, tag="sq")
            nc.vector.scalar_tensor_tensor(
                sq[:], in0=xc[:], scalar=1.0, in1=xc[:],
                op0=ALU.mult, op1=ALU.mult, accum_out=var[:],
            )
            nc.scalar.activation(sd_all[:, m:m + 1], var[:], ACTF.Sqrt,
                                 bias=eps_sb[:], scale=1.0 / D)
            xcs.append(xc)
        nc.vector.reciprocal_approx_fast(rstd_all[:], sd_all[:])
        for m in range(MTI):
            fin = outp.tile([128, D], F32, tag="fin")
            nc.vector.scalar_tensor_tensor(
                fin[:], in0=xcs[m][:], scalar=rstd_all[:, m:m + 1], in1=gam_sb[:],
                op0=ALU.mult, op1=ALU.mult,
            )
            nc.vector.tensor_add(fin[:], fin[:], bet_sb[:])
            nc.sync.dma_start(hout[m * 128:(m + 1) * 128, :], fin[:])
    nc.compile()
    return nc


# --------------------------------------------------------- NEFF-C: lap_smooth
def _build_lap():
    nc = bacc.Bacc("TRN2", target_bir_lowering=False, debug=False, num_devices=NC)
    lt = nc.dram_tensor("lt", [N, RPC], BF16, kind="ExternalInput").ap()
    xf = nc.dram_tensor("xf", [N, D], BF16, kind="ExternalInput").ap()
    xr = nc.dram_tensor("xr", [RPC, D], F32, kind="ExternalInput").ap()
    part = nc.dram_tensor("part", [128, MTI], F32, kind="ExternalOutput").ap()

    with tile.TileContext(nc) as tc, ExitStack() as ctx:
        res = ctx.enter_context(tc.tile_pool(name="res", bufs=1))
        stream = ctx.enter_context(tc.tile_pool(name="stream", bufs=4))
        outp = ctx.enter_context(tc.tile_pool(name="outp", bufs=4))
        psum = ctx.enter_context(tc.tile_pool(name="psum", bufs=1, space="PSUM"))

        xfsb = res.tile([128, KT, D], BF16)
        xf_re = xf.rearrange("(t p) c -> p t c", p=128)
        for c in range(4):
            nc.sync.dma_start(xfsb[:, 8 * c:8 * (c + 1), :], xf_re[:, 8 * c:8 * (c + 1), :])
        xrsb = res.tile([128, MTI, D], F32)
        nc.sync.dma_start(xrsb[:], xr.rearrange("(t p) c -> p t c", p=128))
        pcol = res.tile([128, MTI], F32)

        ps = [psum.tile([128, D], F32, tag=f"ps{m}", name=f"ps{m}") for m in range(MTI)]
        ltts = {}
        for k in range(KT):
            ltt = stream.tile([128, RPC], BF16, tag="ltt", bufs=10, name=f"ltt{k}")
            nc.sync.dma_start(ltt[:], lt[k * 128:(k + 1) * 128, :])
            ltts[k] = ltt
            if k % 8 == 7:
                for kk in range(k - 7, k + 1):
                    for m in range(MTI):
                        nc.tensor.matmul(
                            ps[m][:],
                            lhsT=_mm(ltts[kk][:, m * 128:(m + 1) * 128]),
                            rhs=_mm(xfsb[:, kk, :]),
                            start=(kk == 0),
                            stop=(kk == KT - 1),
                        )
        for m in range(MTI):
            lhs = outp.tile([128, D], F32, tag="lhs")
            nc.vector.tensor_copy(lhs[:], ps[m][:])
            prod = outp.tile([128, D], F32, tag="prod")
            nc.vector.scalar_tensor_tensor(
                prod[:], in0=lhs[:], scalar=1.0, in1=xrsb[:, m, :],
                op0=ALU.mult, op1=ALU.mult, accum_out=pcol[:, m:m + 1],
            )
        nc.sync.dma_start(part[:], pcol[:])
    nc.compile()
    return nc


def _get(name):
    if name not in _cache:
        _cache[name] = {"hop": _build_hop, "main": _build_main, "lap": _build_lap}[name]()
    return _cache[name]


# ------------------------------------------------------------------- driver
def kernel(H, A, adj_mask, L, U, lambda_laps, hop_weights, threshold, gamma, beta):
    H = np.asarray(H, np.float32)
    A = np.asarray(A, np.float32)
    adj_mask = np.asarray(adj_mask, np.float32)
    L = np.asarray(L, np.float32)
    U = np.asarray(U, np.float32)
    lambda_laps = np.asarray(lambda_laps, np.float32)
    hop_weights = np.asarray(hop_weights, np.float32)
    threshold = np.asarray(threshold, np.float32)
    gamma = np.asarray(gamma, np.float32)
    beta = np.asarray(beta, np.float32)

    LAST_EXEC_NS.clear()

    AT = np.ascontiguousarray(A.T)
    LTf = np.ascontiguousarray(L.T)
    MKT = np.ascontiguousarray(adj_mask.T)

    def rows(c):
        return slice(c * RPC, (c + 1) * RPC)

    # ---- L1/L2: hop matmuls
    hop_nc = _get("hop")
    at_c = [np.ascontiguousarray(AT[:, rows(c)]).astype(ml_dtypes.bfloat16)
            for c in range(NC)]
    r1 = _run(hop_nc, [{"at": at_c[c], "x": H.astype(ml_dtypes.bfloat16)}
                       for c in range(NC)], "L1_hop1")
    H1 = np.concatenate([r1[c]["y"] for c in range(NC)], axis=0)
    h1b = H1.astype(ml_dtypes.bfloat16)
    r2 = _run(hop_nc, [{"at": at_c[c], "x": h1b} for c in range(NC)], "L2_hop2")
    H2 = np.concatenate([r2[c]["y"] for c in range(NC)], axis=0)

    # ---- host: small algebra
    w = np.exp(hop_weights - hop_weights.max())
    w = (w / w.sum()).astype(np.float32)
    laps = np.log1p(np.exp(lambda_laps.astype(np.float64))).astype(np.float32)
    coeff = np.float64(R / (N * EPS_SUB ** 2))
    hops = [H, H1, H2]

    qft3 = np.empty((96, N), np.float32)
    yft3 = np.empty((96, N), np.float32)
    qf1s = []
    ukts3 = np.empty((96, D), np.float32)
    ssc = np.zeros((N, D), np.float32)
    for k in range(K_HOPS):
        Yf = hops[k] @ U[k]                       # [N, R]
        G = Yf.T @ Yf
        M = np.eye(R, dtype=np.float64) + coeff * G.astype(np.float64)
        Minv = np.linalg.inv(M).astype(np.float32)
        Qf = Yf @ Minv
        qft3[32 * k:32 * k + 32, :] = Qf.T
        yft3[32 * k:32 * k + 32, :] = Yf.T
        q1 = np.zeros((N, 34), np.float32)
        q1[:, :R] = Qf
        q1[:, R] = 1.0
        qf1s.append(q1)
        ukts3[32 * k:32 * k + 32, :] = (ETA * w[k]) * U[k].T
        ssc -= (ETA * laps[k]) * hops[k]
    thrb = np.broadcast_to(threshold, (128, D)).astype(np.float32).copy()
    gamb = np.broadcast_to(gamma, (128, D)).astype(np.float32).copy()
    betb = np.broadcast_to(beta, (128, D)).astype(np.float32).copy()

    # ---- L3: attention + output rows
    main_nc = _get("main")
    lt_c = [np.ascontiguousarray(LTf[:, rows(c)]).astype(ml_dtypes.bfloat16)
            for c in range(NC)]
    in3 = []
    for c in range(NC):
        m = {
            "maskt": np.ascontiguousarray(MKT[:, rows(c)]).astype(ml_dtypes.bfloat16),
            "lt": lt_c[c],
            "ssc": ssc.astype(ml_dtypes.bfloat16),
            "hr": np.ascontiguousarray(H[rows(c)]),
            "qft3": qft3.astype(ml_dtypes.bfloat16),
            "yrt3": np.ascontiguousarray(yft3[:, rows(c)]).astype(ml_dtypes.bfloat16),
            "ukts3": ukts3,
            "thrb": thrb,
            "nthrb": -thrb,
            "gamb": gamb,
            "betb": betb,
        }
        for k in range(K_HOPS):
            m[f"qf1_{k}"] = qf1s[k].astype(ml_dtypes.bfloat16)
        in3.append(m)
    r3 = _run(main_nc, in3, "L3_main")
    H_out = np.concatenate([r3[c]["hout"] for c in range(NC)], axis=0)

    # ---- L4: lap_smooth partials
    lap_nc = _get("lap")
    hob = H_out.astype(ml_dtypes.bfloat16)
    in4 = [{"lt": np.ascontiguousarray(LTf[:, rows(c)]).astype(ml_dtypes.bfloat16),
            "xf": hob, "xr": np.ascontiguousarray(H_out[rows(c)])}
           for c in range(NC)]
    r4 = _run(lap_nc, in4, "L4_lap")
    lap_smooth = np.float32(sum(float(r4[c]["part"].sum()) for c in range(NC)))

    # ---- host: orth loss (depends only on U input)
    orth = np.float32(0.0)
    I_r = np.eye(R, dtype=np.float32)
    for k in range(K_HOPS):
        for l in range(k, K_HOPS):
            Gkl = U[k].T @ U[l]
            Gkl = Gkl - I_r if l == k else Gkl
            orth = orth + np.float32((Gkl ** 2).sum())

    return H_out, orth, lap_smooth, w


# revision 24
# speedup vs baseline: 1.0656x; 1.0656x over previous
"""Trainium2 Bass kernel for nn_F1Layer (gnn_message_passing).

Row-shard n=4096 across 8 NeuronCores (512 rows/core). Four SPMD launches
(3 distinct NEFFs), zero device collectives — cross-core exchanges ride host
gathers between launches (cheaper than the ~70-100us ncfw collective cold
cost per NEFF):

  L1 (NEFF-A): H1_rows = A_rows @ H          (host gathers H1_full)
  L2 (NEFF-A): H2_rows = A_rows @ H1_full    (host gathers H2_full)
  host:        Yk = Hk@Uk, Gram, M_inv, Qf/QfT, scaled laplacian source —
               all tiny (O(n r) / O(r^3))
  L3 (NEFF-B): per-hop subspace attention with scores computed TRANSPOSED so
               the softmax contraction lands on the PE partition axis, masked
               exp, row-normalized aggregation, laplacian term, soft-threshold
               + residual + layernorm -> H_out rows (host gathers H_out)
  L4 (NEFF-C): lap_smooth partials = rowsum(H_out_r * (L_r @ H_out_full))

All matmuls use natural (host-pre-transposed) layouts; no device transposes.
The three hops' K=32 matmuls are packed at partition bases 0/32/64 so they
run concurrently in distinct PE row-groups.
"""

import os
import ml_dtypes
import numpy as np
from contextlib import ExitStack

import concourse.bass as bass
import concourse.tile as tile
from concourse import bacc, mybir
from concourse.bass_utils import run_bass_kernel_spmd

N = 4096
D = 256
R = 32
K_HOPS = 3
NC = 8
RPC = N // NC          # rows per core = 512
KT = N // 128          # 32 contraction tiles
MTI = RPC // 128       # 4 row tiles per core
ETA = np.float32(0.5)
EPS_SUB = 0.5
LN_EPS = np.float32(1e-5)

F32 = mybir.dt.float32
F32R = mybir.dt.float32r
BF16 = mybir.dt.bfloat16
AX = mybir.AxisListType
ALU = mybir.AluOpType
ACTF = mybir.ActivationFunctionType

# matmul input dtype knob: "f32r" (fast fp32 path) or "f32"
MM_MODE = os.environ.get("KERNEL_MM_MODE", "f32r")
MDT = F32R if MM_MODE == "f32r" else F32

TRACE = False            # set by test harness only
LAST_EXEC_NS = []        # per-launch (label, exec_ns) when TRACE

_cache = {}


def _mm(ap):
    return ap


def _run(nc, in_maps, label):
    res = run_bass_kernel_spmd(nc, in_maps, core_ids=list(range(NC)), trace=TRACE)
    if TRACE:
        LAST_EXEC_NS.append((label, res.exec_time_ns))
    return res.results


# ---------------------------------------------------------------- NEFF-A: hop
def _build_hop():
    """y[512,256] = (at)^T[512,4096] @ x[4096,256] per core."""
    nc = bacc.Bacc("TRN2", target_bir_lowering=False, debug=False, num_devices=NC)
    at = nc.dram_tensor("at", [N, RPC], BF16, kind="ExternalInput").ap()
    x = nc.dram_tensor("x", [N, D], BF16, kind="ExternalInput").ap()
    y = nc.dram_tensor("y", [RPC, D], F32, kind="ExternalOutput").ap()

    with tile.TileContext(nc) as tc, ExitStack() as ctx:
        res = ctx.enter_context(tc.tile_pool(name="res", bufs=1))
        stream = ctx.enter_context(tc.tile_pool(name="stream", bufs=4))
        outp = ctx.enter_context(tc.tile_pool(name="outp", bufs=4))
        psum = ctx.enter_context(tc.tile_pool(name="psum", bufs=1, space="PSUM"))

        xsb = res.tile([128, KT, D], BF16)
        x_re = x.rearrange("(t p) c -> p t c", p=128)
        nc.sync.dma_start(xsb[:, 0:8, :], x_re[:, 0:8, :])

        ps = [psum.tile([128, D], F32, tag=f"ps{m}", name=f"ps{m}") for m in range(MTI)]
        atts = {}
        for k in range(KT):
            att = stream.tile([128, RPC], BF16, tag="att", bufs=10, name=f"att{k}")
            nc.sync.dma_start(att[:], at[k * 128:(k + 1) * 128, :])
            atts[k] = att
            if k % 8 == 0 and k < 24:
                c = k // 8 + 1
                nc.sync.dma_start(xsb[:, 8 * c:8 * (c + 1), :], x_re[:, 8 * c:8 * (c + 1), :])
            # drain matmuls in 8-kt dense bursts so the PE crosses the HAM
            # warm window instead of idling between per-kt DMA waits
            if k % 8 == 7:
                for kk in range(k - 7, k + 1):
                    for m in range(MTI):
                        nc.tensor.matmul(
                            ps[m][:],
                            lhsT=_mm(atts[kk][:, m * 128:(m + 1) * 128]),
                            rhs=_mm(xsb[:, kk, :]),
                            start=(kk == 0),
                            stop=(kk == KT - 1),
                        )
        for m in range(MTI):
            ot = outp.tile([128, D], F32)
            nc.vector.tensor_copy(ot[:], ps[m][:])
            nc.sync.dma_start(y[m * 128:(m + 1) * 128, :], ot[:])
    nc.compile()
    return nc


# ----------------------------------------------------------- NEFF-B: attention
def _build_main():
    nc = bacc.Bacc("TRN2", target_bir_lowering=False, debug=False, num_devices=NC)
    maskt = nc.dram_tensor("maskt", [N, RPC], BF16, kind="ExternalInput").ap()
    lt = nc.dram_tensor("lt", [N, RPC], BF16, kind="ExternalInput").ap()
    ssc = nc.dram_tensor("ssc", [N, D], BF16, kind="ExternalInput").ap()  # -eta*sum(lap_k*Hk)
    hr = nc.dram_tensor("hr", [RPC, D], F32, kind="ExternalInput").ap()
    qft3 = nc.dram_tensor("qft3", [96, N], BF16, kind="ExternalInput").ap()   # hop k at parts 32k
    yrt3 = nc.dram_tensor("yrt3", [96, RPC], BF16, kind="ExternalInput").ap()
    qf1s = [nc.dram_tensor(f"qf1_{k}", [N, 34], BF16, kind="ExternalInput").ap()
            for k in range(K_HOPS)]
    ukts3 = nc.dram_tensor("ukts3", [96, D], MDT, kind="ExternalInput").ap()  # eta*w_k*Uk^T
    thrb = nc.dram_tensor("thrb", [128, D], F32, kind="ExternalInput").ap()
    nthrb = nc.dram_tensor("nthrb", [128, D], F32, kind="ExternalInput").ap()
    gamb = nc.dram_tensor("gamb", [128, D], F32, kind="ExternalInput").ap()
    betb = nc.dram_tensor("betb", [128, D], F32, kind="ExternalInput").ap()
    hout = nc.dram_tensor("hout", [RPC, D], F32, kind="ExternalOutput").ap()

    with tile.TileContext(nc) as tc, ExitStack() as ctx:
        res = ctx.enter_context(tc.tile_pool(name="res", bufs=1))
        stream = ctx.enter_context(tc.tile_pool(name="stream", bufs=4))
        tbuf = ctx.enter_context(tc.tile_pool(name="tbuf", bufs=4))
        small = ctx.enter_context(tc.tile_pool(name="small", bufs=1))
        outp = ctx.enter_context(tc.tile_pool(name="outp", bufs=4))
        psum_s = ctx.enter_context(tc.tile_pool(name="psum_s", bufs=1, space="PSUM"))
        psum_u = ctx.enter_context(tc.tile_pool(name="psum_u", bufs=1, space="PSUM"))

        # ---- PE warm-up: dependency-free dense matmuls cross the HAM busy
        # window while the first input DMAs are still in flight
        wu = res.tile([128, RPC], BF16)
        nc.vector.memset(wu[:], 0.0)
        for i in range(16):
            wps = psum_s.tile([128, RPC], F32, tag="sp", bufs=5, name=f"wps{i}")
            nc.tensor.matmul(wps[:], lhsT=_mm(wu[:, 0:128]), rhs=_mm(wu[:]),
                             start=True, stop=True)

        # ---- residents needed by C1 first, so its matmuls start immediately
        ssc_sb = res.tile([128, KT, D], BF16)
        ssc_re = ssc.rearrange("(t p) c -> p t c", p=128)
        for c in range(8):
            nc.sync.dma_start(ssc_sb[:, 4 * c:4 * (c + 1), :], ssc_re[:, 4 * c:4 * (c + 1), :])

        # ---- C1: x0 = -eta * L @ (sum lap_k Hk) + Hr  per i-block
        # (matmuls drained in 8-kt dense bursts to keep HAM warm)
        x0 = [res.tile([128, D], F32, tag=f"x0_{m}", name=f"x0_{m}") for m in range(MTI)]
        acc1 = [psum_s.tile([128, D], F32, tag="sp", bufs=5, name=f"acc1_{m}")
                for m in range(MTI)]
        ltts_c1 = {}
        for kt in range(KT):
            ltt = stream.tile([128, RPC], BF16, tag="lt", bufs=10, name=f"ltc{kt}")
            nc.sync.dma_start(ltt[:], lt[kt * 128:(kt + 1) * 128, :])
            ltts_c1[kt] = ltt
            if kt % 8 == 7:
                for kk in range(kt - 7, kt + 1):
                    for m in range(MTI):
                        nc.tensor.matmul(
                            acc1[m][:],
                            lhsT=_mm(ltts_c1[kk][:, m * 128:(m + 1) * 128]),
                            rhs=_mm(ssc_sb[:, kk, :]),
                            start=(kk == 0),
                            stop=(kk == KT - 1),
                        )
        hr_sb = res.tile([128, MTI, D], F32)
        nc.sync.dma_start(hr_sb[:], hr.rearrange("(t p) c -> p t c", p=128))
        for m in range(MTI):
            nc.vector.tensor_add(x0[m][:], acc1[m][:], hr_sb[:, m, :])

        # ---- attention residents (needed ~25us in, after C1 is streaming)
        qft_sb = res.tile([96, N], BF16)
        nc.sync.dma_start(qft_sb[:], qft3[:])
        yrt_sb = res.tile([96, RPC], BF16)
        nc.sync.dma_start(yrt_sb[:], yrt3[:])
        ukts_sb = res.tile([96, D], MDT)
        nc.sync.dma_start(ukts_sb[:], ukts3[:])
        qf1_sb = [res.tile([128, KT, 34], BF16, tag=f"qf1_{k}", name=f"qf1sb{k}") for k in range(K_HOPS)]
        for k in range(K_HOPS):
            nc.sync.dma_start(qf1_sb[k][:], qf1s[k].rearrange("(t p) c -> p t c", p=128))
        thr_sb = res.tile([128, D], F32)
        nc.sync.dma_start(thr_sb[:], thrb[:])
        nthr_sb = res.tile([128, D], F32)
        nc.sync.dma_start(nthr_sb[:], nthrb[:])
        gam_sb = res.tile([128, D], F32)
        nc.sync.dma_start(gam_sb[:], gamb[:])
        bet_sb = res.tile([128, D], F32)
        nc.sync.dma_start(bet_sb[:], betb[:])
        eps_sb = res.tile([128, 1], F32)
        nc.vector.memset(eps_sb[:], float(LN_EPS))

        # ---- A: per j-tile, all hops: scores^T -> exp -> mask -> u accumulation
        # (u-matmuls software-pipelined one j-tile behind the scores)
        ups = [psum_u.tile([34, RPC], F32, tag=f"u{k}", name=f"ups{k}") for k in range(K_HOPS)]
        pend = []          # (jt, [tm0, tm1, tm2]) not yet fed to the u-matmuls

        def drain_u(last):
            for pjt, tms in pend:
                for k in range(K_HOPS):
                    nc.tensor.matmul(
                        ups[k][:],
                        lhsT=_mm(qf1_sb[k][:, pjt, :]),
                        rhs=_mm(tms[k][:]),
                        start=(pjt == 0),
                        stop=(last and pjt == KT - 1),
                    )
            pend.clear()

        for jt in range(KT):
            mkt = stream.tile([128, RPC], BF16, tag="mask")
            nc.sync.dma_start(mkt[:], maskt[jt * 128:(jt + 1) * 128, :])
            cur_tm = []
            for k in range(K_HOPS):
                sp = psum_s.tile([128, RPC], F32, tag="sp", bufs=5)
                nc.tensor.matmul(
                    sp[:],
                    lhsT=_mm(qft_sb[32 * k:32 * k + 32, jt * 128:(jt + 1) * 128]),
                    rhs=_mm(yrt_sb[32 * k:32 * k + 32, :]),
                    start=True,
                    stop=True,
                )
                tt = tbuf.tile([128, RPC], BF16, tag="tt", bufs=8)
                nc.scalar.activation(tt[:], sp[:], ACTF.Exp)
                tm = tbuf.tile([128, RPC], BF16, tag="tm", bufs=16)
                nc.vector.tensor_mul(tm[:], tt[:], mkt[:])
                cur_tm.append(tm)
            pend.append((jt, cur_tm))
            # every 4 j-tiles, drain the pending u-matmuls as one dense
            # 12-matmul PE burst (long enough to cross the HAM warm window)
            if len(pend) == 4 and jt != KT - 1:
                drain_u(last=False)
        drain_u(last=True)

        # ---- B: vT = (u / rowsum), hop k at partitions 32k (one batched recip)
        vt_sb = small.tile([96, RPC], MDT)
        for k in range(K_HOPS):
            srow = small.tile([1, RPC], F32, tag=f"srow{k}", name=f"srow{k}")
            nc.vector.tensor_copy(srow[:], ups[k][32:33, :])
            rec = small.tile([1, RPC], F32, tag=f"rec{k}", name=f"rec{k}")
            nc.vector.reciprocal_approx_fast(rec[:], srow[:])
            bc = small.tile([32, RPC], F32, tag=f"bc{k}", name=f"bc{k}")
            nc.gpsimd.partition_broadcast(bc[:], rec[:])
            nc.vector.tensor_mul(vt_sb[32 * k:32 * k + 32, :], ups[k][0:32, :], bc[:])

        # ---- C2: g accumulation + epilogue per i-block
        acc2 = [psum_s.tile([128, D], F32, tag="sp", bufs=5, name=f"acc2_{m}")
                for m in range(MTI)]
        for m in range(MTI):
            for k in range(K_HOPS):
                nc.tensor.matmul(
                    acc2[m][:],
                    lhsT=_mm(vt_sb[32 * k:32 * k + 32, m * 128:(m + 1) * 128]),
                    rhs=_mm(ukts_sb[32 * k:32 * k + 32, :]),
                    start=(k == 0),
                    stop=(k == K_HOPS - 1),
                )
        sd_all = small.tile([128, MTI], F32)
        rstd_all = small.tile([128, MTI], F32)
        xcs = []
        for m in range(MTI):
            x = outp.tile([128, D], F32, tag="x")
            nc.vector.tensor_add(x[:], acc2[m][:], x0[m][:])          # H_half
            cl = outp.tile([128, D], F32, tag="cl")
            nc.vector.tensor_max(cl[:], x[:], nthr_sb[:])
            nc.vector.tensor_tensor(cl[:], cl[:], thr_sb[:], ALU.min)
            y2 = outp.tile([128, D], F32, tag="y2")
            nc.vector.tensor_sub(y2[:], x[:], cl[:])             # soft-threshold
            nc.vector.tensor_add(y2[:], y2[:], hr_sb[:, m, :])   # residual
            # layernorm stats
            red = outp.tile([128, 1], F32, tag="red")
            nc.vector.tensor_reduce(red[:], y2[:], AX.X, ALU.add)
            mu = outp.tile([128, 1], F32, tag="mu")
            nc.scalar.mul(mu[:], red[:], 1.0 / D)
            xc = outp.tile([128, D], F32, tag=f"xc{m}", name=f"xc{m}")
            nc.vector.tensor_scalar_sub(xc[:], y2[:], mu[:])
            var = outp.tile([128, 1], F32, tag="var")
            sq = outp.tile([128, D], F32, tag="sq")
            nc.vector.scalar_tensor_tensor(
                sq[:], in0=xc[:], scalar=1.0, in1=xc[:],
                op0=ALU.mult, op1=ALU.mult, accum_out=var[:],
            )
            nc.scalar.activation(sd_all[:, m:m + 1], var[:], ACTF.Sqrt,
                                 bias=eps_sb[:], scale=1.0 / D)
            xcs.append(xc)
        nc.vector.reciprocal_approx_fast(rstd_all[:], sd_all[:])
        for m in range(MTI):
            fin = outp.tile([128, D], F32, tag="fin")
            nc.vector.scalar_tensor_tensor(
                fin[:], in0=xcs[m][:], scalar=rstd_all[:, m:m + 1], in1=gam_sb[:],
                op0=ALU.mult, op1=ALU.mult,
            )
            nc.vector.tensor_add(fin[:], fin[:], bet_sb[:])
            nc.sync.dma_start(hout[m * 128:(m + 1) * 128, :], fin[:])
    nc.compile()
    return nc


# --------------------------------------------------------- NEFF-C: lap_smooth
def _build_lap():
    nc = bacc.Bacc("TRN2", target_bir_lowering=False, debug=False, num_devices=NC)
    lt = nc.dram_tensor("lt", [N, RPC], BF16, kind="ExternalInput").ap()
    xf = nc.dram_tensor("xf", [N, D], BF16, kind="ExternalInput").ap()
    xr = nc.dram_tensor("xr", [RPC, D], F32, kind="ExternalInput").ap()
    part = nc.dram_tensor("part", [128, MTI], F32, kind="ExternalOutput").ap()

    with tile.TileContext(nc) as tc, ExitStack() as ctx:
        res = ctx.enter_context(tc.tile_pool(name="res", bufs=1))
        stream = ctx.enter_context(tc.tile_pool(name="stream", bufs=4))
        outp = ctx.enter_context(tc.tile_pool(name="outp", bufs=4))
        psum = ctx.enter_context(tc.tile_pool(name="psum", bufs=1, space="PSUM"))

        xfsb = res.tile([128, KT, D], BF16)
        xf_re = xf.rearrange("(t p) c -> p t c", p=128)
        nc.sync.dma_start(xfsb[:, 0:8, :], xf_re[:, 0:8, :])
        xrsb = res.tile([128, MTI, D], F32)
        nc.sync.dma_start(xrsb[:], xr.rearrange("(t p) c -> p t c", p=128))
        pcol = res.tile([128, MTI], F32)

        ps = [psum.tile([128, D], F32, tag=f"ps{m}", name=f"ps{m}") for m in range(MTI)]
        ltts = {}
        for k in range(KT):
            ltt = stream.tile([128, RPC], BF16, tag="ltt", bufs=10, name=f"ltt{k}")
            nc.sync.dma_start(ltt[:], lt[k * 128:(k + 1) * 128, :])
            ltts[k] = ltt
            if k % 8 == 0 and k < 24:
                c = k // 8 + 1
                nc.sync.dma_start(xfsb[:, 8 * c:8 * (c + 1), :], xf_re[:, 8 * c:8 * (c + 1), :])
            if k % 8 == 7:
                for kk in range(k - 7, k + 1):
                    for m in range(MTI):
                        nc.tensor.matmul(
                            ps[m][:],
                            lhsT=_mm(ltts[kk][:, m * 128:(m + 1) * 128]),
                            rhs=_mm(xfsb[:, kk, :]),
                            start=(kk == 0),
                            stop=(kk == KT - 1),
                        )
        for m in range(MTI):
            lhs = outp.tile([128, D], F32, tag="lhs")
            nc.vector.tensor_copy(lhs[:], ps[m][:])
            prod = outp.tile([128, D], F32, tag="prod")
            nc.vector.scalar_tensor_tensor(
                prod[:], in0=lhs[:], scalar=1.0, in1=xrsb[:, m, :],
                op0=ALU.mult, op1=ALU.mult, accum_out=pcol[:, m:m + 1],
            )
        nc.sync.dma_start(part[:], pcol[:])
    nc.compile()
    return nc


def _get(name):
    if name not in _cache:
        _cache[name] = {"hop": _build_hop, "main": _build_main, "lap": _build_lap}[name]()
    return _cache[name]


# ------------------------------------------------------------------- driver
def kernel(H, A, adj_mask, L, U, lambda_laps, hop_weights, threshold, gamma, beta):
    H = np.asarray(H, np.float32)
    A = np.asarray(A, np.float32)
    adj_mask = np.asarray(adj_mask, np.float32)
    L = np.asarray(L, np.float32)
    U = np.asarray(U, np.float32)
    lambda_laps = np.asarray(lambda_laps, np.float32)
    hop_weights = np.asarray(hop_weights, np.float32)
    threshold = np.asarray(threshold, np.float32)
    gamma = np.asarray(gamma, np.float32)
    beta = np.asarray(beta, np.float32)

    LAST_EXEC_NS.clear()

    AT = np.ascontiguousarray(A.T)
    LTf = np.ascontiguousarray(L.T)
    MKT = np.ascontiguousarray(adj_mask.T)

    def rows(c):
        return slice(c * RPC, (c + 1) * RPC)

    # ---- L1/L2: hop matmuls
    hop_nc = _get("hop")
    at_c = [np.ascontiguousarray(AT[:, rows(c)]).astype(ml_dtypes.bfloat16)
            for c in range(NC)]
    r1 = _run(hop_nc, [{"at": at_c[c], "x": H.astype(ml_dtypes.bfloat16)}
                       for c in range(NC)], "L1_hop1")
    H1 = np.concatenate([r1[c]["y"] for c in range(NC)], axis=0)
    h1b = H1.astype(ml_dtypes.bfloat16)
    r2 = _run(hop_nc, [{"at": at_c[c], "x": h1b} for c in range(NC)], "L2_hop2")
    H2 = np.concatenate([r2[c]["y"] for c in range(NC)], axis=0)

    # ---- host: small algebra
    w = np.exp(hop_weights - hop_weights.max())
    w = (w / w.sum()).astype(np.float32)
    laps = np.log1p(np.exp(lambda_laps.astype(np.float64))).astype(np.float32)
    coeff = np.float64(R / (N * EPS_SUB ** 2))
    hops = [H, H1, H2]

    qft3 = np.empty((96, N), np.float32)
    yft3 = np.empty((96, N), np.float32)
    qf1s = []
    ukts3 = np.empty((96, D), np.float32)
    ssc = np.zeros((N, D), np.float32)
    for k in range(K_HOPS):
        Yf = hops[k] @ U[k]                       # [N, R]
        G = Yf.T @ Yf
        M = np.eye(R, dtype=np.float64) + coeff * G.astype(np.float64)
        Minv = np.linalg.inv(M).astype(np.float32)
        Qf = Yf @ Minv
        qft3[32 * k:32 * k + 32, :] = Qf.T
        yft3[32 * k:32 * k + 32, :] = Yf.T
        q1 = np.zeros((N, 34), np.float32)
        q1[:, :R] = Qf
        q1[:, R] = 1.0
        qf1s.append(q1)
        ukts3[32 * k:32 * k + 32, :] = (ETA * w[k]) * U[k].T
        ssc -= (ETA * laps[k]) * hops[k]
    thrb = np.broadcast_to(threshold, (128, D)).astype(np.float32).copy()
    gamb = np.broadcast_to(gamma, (128, D)).astype(np.float32).copy()
    betb = np.broadcast_to(beta, (128, D)).astype(np.float32).copy()

    # ---- L3: attention + output rows
    main_nc = _get("main")
    lt_c = [np.ascontiguousarray(LTf[:, rows(c)]).astype(ml_dtypes.bfloat16)
            for c in range(NC)]
    in3 = []
    for c in range(NC):
        m = {
            "maskt": np.ascontiguousarray(MKT[:, rows(c)]).astype(ml_dtypes.bfloat16),
            "lt": lt_c[c],
            "ssc": ssc.astype(ml_dtypes.bfloat16),
            "hr": np.ascontiguousarray(H[rows(c)]),
            "qft3": qft3.astype(ml_dtypes.bfloat16),
            "yrt3": np.ascontiguousarray(yft3[:, rows(c)]).astype(ml_dtypes.bfloat16),
            "ukts3": ukts3,
            "thrb": thrb,
            "nthrb": -thrb,
            "gamb": gamb,
            "betb": betb,
        }
        for k in range(K_HOPS):
            m[f"qf1_{k}"] = qf1s[k].astype(ml_dtypes.bfloat16)
        in3.append(m)
    r3 = _run(main_nc, in3, "L3_main")
    H_out = np.concatenate([r3[c]["hout"] for c in range(NC)], axis=0)

    # ---- L4: lap_smooth partials
    lap_nc = _get("lap")
    hob = H_out.astype(ml_dtypes.bfloat16)
    in4 = [{"lt": np.ascontiguousarray(LTf[:, rows(c)]).astype(ml_dtypes.bfloat16),
            "xf": hob, "xr": np.ascontiguousarray(H_out[rows(c)])}
           for c in range(NC)]
    r4 = _run(lap_nc, in4, "L4_lap")
    lap_smooth = np.float32(sum(float(r4[c]["part"].sum()) for c in range(NC)))

    # ---- host: orth loss (depends only on U input)
    orth = np.float32(0.0)
    I_r = np.eye(R, dtype=np.float32)
    for k in range(K_HOPS):
        for l in range(k, K_HOPS):
            Gkl = U[k].T @ U[l]
            Gkl = Gkl - I_r if l == k else Gkl
            orth = orth + np.float32((Gkl ** 2).sum())

    return H_out, orth, lap_smooth, w


# revision 25
# speedup vs baseline: 1.0846x; 1.0178x over previous
"""Trainium2 Bass kernel for nn_F1Layer (gnn_message_passing).

Row-shard n=4096 across 8 NeuronCores (512 rows/core). Four SPMD launches
(3 distinct NEFFs), zero device collectives — cross-core exchanges ride host
gathers between launches (cheaper than the ~70-100us ncfw collective cold
cost per NEFF):

  L1 (NEFF-A): H1_rows = A_rows @ H          (host gathers H1_full)
  L2 (NEFF-A): H2_rows = A_rows @ H1_full    (host gathers H2_full)
  host:        Yk = Hk@Uk, Gram, M_inv, Qf/QfT, scaled laplacian source —
               all tiny (O(n r) / O(r^3))
  L3 (NEFF-B): per-hop subspace attention with scores computed TRANSPOSED so
               the softmax contraction lands on the PE partition axis, masked
               exp, row-normalized aggregation, laplacian term, soft-threshold
               + residual + layernorm -> H_out rows (host gathers H_out)
  L4 (NEFF-C): lap_smooth partials = rowsum(H_out_r * (L_r @ H_out_full))

All matmuls use natural (host-pre-transposed) layouts; no device transposes.
The three hops' K=32 matmuls are packed at partition bases 0/32/64 so they
run concurrently in distinct PE row-groups.
"""

import os
import ml_dtypes
import numpy as np
from contextlib import ExitStack

import concourse.bass as bass
import concourse.tile as tile
from concourse import bacc, mybir
from concourse.bass_utils import run_bass_kernel_spmd

N = 4096
D = 256
R = 32
K_HOPS = 3
NC = 8
RPC = N // NC          # rows per core = 512
KT = N // 128          # 32 contraction tiles
MTI = RPC // 128       # 4 row tiles per core
ETA = np.float32(0.5)
EPS_SUB = 0.5
LN_EPS = np.float32(1e-5)

F32 = mybir.dt.float32
F32R = mybir.dt.float32r
BF16 = mybir.dt.bfloat16
AX = mybir.AxisListType
ALU = mybir.AluOpType
ACTF = mybir.ActivationFunctionType

# matmul input dtype knob: "f32r" (fast fp32 path) or "f32"
MM_MODE = os.environ.get("KERNEL_MM_MODE", "f32r")
MDT = F32R if MM_MODE == "f32r" else F32

TRACE = False            # set by test harness only
LAST_EXEC_NS = []        # per-launch (label, exec_ns) when TRACE

_cache = {}


def _mm(ap):
    return ap


def _run(nc, in_maps, label):
    res = run_bass_kernel_spmd(nc, in_maps, core_ids=list(range(NC)), trace=TRACE)
    if TRACE:
        LAST_EXEC_NS.append((label, res.exec_time_ns))
    return res.results


# ---------------------------------------------------------------- NEFF-A: hop
def _build_hop():
    """y[512,256] = (at)^T[512,4096] @ x[4096,256] per core."""
    nc = bacc.Bacc("TRN2", target_bir_lowering=False, debug=False, num_devices=NC)
    at = nc.dram_tensor("at", [N, RPC], BF16, kind="ExternalInput").ap()
    x = nc.dram_tensor("x", [N, D], BF16, kind="ExternalInput").ap()
    y = nc.dram_tensor("y", [RPC, D], F32, kind="ExternalOutput").ap()

    with tile.TileContext(nc) as tc, ExitStack() as ctx:
        res = ctx.enter_context(tc.tile_pool(name="res", bufs=1))
        stream = ctx.enter_context(tc.tile_pool(name="stream", bufs=4))
        outp = ctx.enter_context(tc.tile_pool(name="outp", bufs=4))
        psum = ctx.enter_context(tc.tile_pool(name="psum", bufs=1, space="PSUM"))

        xsb = res.tile([128, KT, D], BF16)
        x_re = x.rearrange("(t p) c -> p t c", p=128)
        nc.sync.dma_start(xsb[:, 0:8, :], x_re[:, 0:8, :])

        ps = [psum.tile([128, D], F32, tag=f"ps{m}", name=f"ps{m}") for m in range(MTI)]
        atts = {}
        for k in range(KT):
            att = stream.tile([128, RPC], BF16, tag="att", bufs=10, name=f"att{k}")
            nc.sync.dma_start(att[:], at[k * 128:(k + 1) * 128, :])
            atts[k] = att
            if k % 8 == 0 and k < 24:
                c = k // 8 + 1
                nc.sync.dma_start(xsb[:, 8 * c:8 * (c + 1), :], x_re[:, 8 * c:8 * (c + 1), :])
            # drain matmuls in 8-kt dense bursts so the PE crosses the HAM
            # warm window instead of idling between per-kt DMA waits
            if k % 8 == 7:
                for kk in range(k - 7, k + 1):
                    for m in range(MTI):
                        nc.tensor.matmul(
                            ps[m][:],
                            lhsT=_mm(atts[kk][:, m * 128:(m + 1) * 128]),
                            rhs=_mm(xsb[:, kk, :]),
                            start=(kk == 0),
                            stop=(kk == KT - 1),
                        )
        for m in range(MTI):
            ot = outp.tile([128, D], F32)
            nc.vector.tensor_copy(ot[:], ps[m][:])
            nc.sync.dma_start(y[m * 128:(m + 1) * 128, :], ot[:])
    nc.compile()
    return nc


# ----------------------------------------------------------- NEFF-B: attention
def _build_main():
    nc = bacc.Bacc("TRN2", target_bir_lowering=False, debug=False, num_devices=NC)
    maskt = nc.dram_tensor("maskt", [N, RPC], BF16, kind="ExternalInput").ap()
    lt = nc.dram_tensor("lt", [N, RPC], BF16, kind="ExternalInput").ap()
    ssc = nc.dram_tensor("ssc", [N, D], BF16, kind="ExternalInput").ap()  # -eta*sum(lap_k*Hk)
    hr = nc.dram_tensor("hr", [RPC, D], F32, kind="ExternalInput").ap()
    qft3 = nc.dram_tensor("qft3", [96, N], BF16, kind="ExternalInput").ap()   # hop k at parts 32k
    yrt3 = nc.dram_tensor("yrt3", [96, RPC], BF16, kind="ExternalInput").ap()
    qf1s = [nc.dram_tensor(f"qf1_{k}", [N, 34], BF16, kind="ExternalInput").ap()
            for k in range(K_HOPS)]
    ukts3 = nc.dram_tensor("ukts3", [96, D], MDT, kind="ExternalInput").ap()  # eta*w_k*Uk^T
    thrb = nc.dram_tensor("thrb", [128, D], F32, kind="ExternalInput").ap()
    nthrb = nc.dram_tensor("nthrb", [128, D], F32, kind="ExternalInput").ap()
    gamb = nc.dram_tensor("gamb", [128, D], F32, kind="ExternalInput").ap()
    betb = nc.dram_tensor("betb", [128, D], F32, kind="ExternalInput").ap()
    hout = nc.dram_tensor("hout", [RPC, D], F32, kind="ExternalOutput").ap()

    with tile.TileContext(nc) as tc, ExitStack() as ctx:
        res = ctx.enter_context(tc.tile_pool(name="res", bufs=1))
        stream = ctx.enter_context(tc.tile_pool(name="stream", bufs=4))
        tbuf = ctx.enter_context(tc.tile_pool(name="tbuf", bufs=4))
        small = ctx.enter_context(tc.tile_pool(name="small", bufs=1))
        outp = ctx.enter_context(tc.tile_pool(name="outp", bufs=4))
        psum_s = ctx.enter_context(tc.tile_pool(name="psum_s", bufs=1, space="PSUM"))
        psum_u = ctx.enter_context(tc.tile_pool(name="psum_u", bufs=1, space="PSUM"))

        # ---- PE warm-up: dependency-free dense matmuls cross the HAM busy
        # window while the first input DMAs are still in flight
        wu = res.tile([128, RPC], BF16)
        nc.vector.memset(wu[:], 0.0)
        for i in range(16):
            wps = psum_s.tile([128, RPC], F32, tag="sp", bufs=5, name=f"wps{i}")
            nc.tensor.matmul(wps[:], lhsT=_mm(wu[:, 0:128]), rhs=_mm(wu[:]),
                             start=True, stop=True)

        # ---- residents needed by C1 first, so its matmuls start immediately
        ssc_sb = res.tile([128, KT, D], BF16)
        ssc_re = ssc.rearrange("(t p) c -> p t c", p=128)
        for c in range(8):
            nc.sync.dma_start(ssc_sb[:, 4 * c:4 * (c + 1), :], ssc_re[:, 4 * c:4 * (c + 1), :])

        # ---- attention residents early (they are small; C1's lt stream is
        # long, and the A phase stalls if these land late)
        qft_sb = res.tile([96, N], BF16)
        nc.sync.dma_start(qft_sb[:], qft3[:])
        yrt_sb = res.tile([96, RPC], BF16)
        nc.sync.dma_start(yrt_sb[:], yrt3[:])
        qf1_sb = [res.tile([128, KT, 34], BF16, tag=f"qf1_{k}", name=f"qf1sb{k}") for k in range(K_HOPS)]
        for k in range(K_HOPS):
            nc.sync.dma_start(qf1_sb[k][:], qf1s[k].rearrange("(t p) c -> p t c", p=128))

        # ---- C1: x0 = -eta * L @ (sum lap_k Hk) + Hr  per i-block
        # (matmuls drained in 8-kt dense bursts to keep HAM warm)
        x0 = [res.tile([128, D], F32, tag=f"x0_{m}", name=f"x0_{m}") for m in range(MTI)]
        acc1 = [psum_s.tile([128, D], F32, tag="sp", bufs=5, name=f"acc1_{m}")
                for m in range(MTI)]
        ltts_c1 = {}
        for kt in range(KT):
            ltt = stream.tile([128, RPC], BF16, tag="lt", bufs=10, name=f"ltc{kt}")
            nc.sync.dma_start(ltt[:], lt[kt * 128:(kt + 1) * 128, :])
            ltts_c1[kt] = ltt
            if kt % 8 == 7:
                for kk in range(kt - 7, kt + 1):
                    for m in range(MTI):
                        nc.tensor.matmul(
                            acc1[m][:],
                            lhsT=_mm(ltts_c1[kk][:, m * 128:(m + 1) * 128]),
                            rhs=_mm(ssc_sb[:, kk, :]),
                            start=(kk == 0),
                            stop=(kk == KT - 1),
                        )
        hr_sb = res.tile([128, MTI, D], F32)
        nc.sync.dma_start(hr_sb[:], hr.rearrange("(t p) c -> p t c", p=128))
        for m in range(MTI):
            nc.vector.tensor_add(x0[m][:], acc1[m][:], hr_sb[:, m, :])

        # ---- remaining residents (needed only at C2/epilogue)
        ukts_sb = res.tile([96, D], MDT)
        nc.sync.dma_start(ukts_sb[:], ukts3[:])
        thr_sb = res.tile([128, D], F32)
        nc.sync.dma_start(thr_sb[:], thrb[:])
        nthr_sb = res.tile([128, D], F32)
        nc.sync.dma_start(nthr_sb[:], nthrb[:])
        gam_sb = res.tile([128, D], F32)
        nc.sync.dma_start(gam_sb[:], gamb[:])
        bet_sb = res.tile([128, D], F32)
        nc.sync.dma_start(bet_sb[:], betb[:])
        eps_sb = res.tile([128, 1], F32)
        nc.vector.memset(eps_sb[:], float(LN_EPS))

        # ---- A: per j-tile, all hops: scores^T -> exp -> mask -> u accumulation
        # (u-matmuls software-pipelined one j-tile behind the scores)
        ups = [psum_u.tile([34, RPC], F32, tag=f"u{k}", name=f"ups{k}") for k in range(K_HOPS)]
        pend = []          # (jt, [tm0, tm1, tm2]) not yet fed to the u-matmuls

        def drain_u(last):
            for pjt, tms in pend:
                for k in range(K_HOPS):
                    nc.tensor.matmul(
                        ups[k][:],
                        lhsT=_mm(qf1_sb[k][:, pjt, :]),
                        rhs=_mm(tms[k][:]),
                        start=(pjt == 0),
                        stop=(last and pjt == KT - 1),
                    )
            pend.clear()

        for jt in range(KT):
            if jt % 8 == 0:
                # dependency-free dense burst to re-arm the HAM clock-gate
                # (the data-dependent bursts fragment and leave PE throttled)
                for i in range(10):
                    wps = psum_s.tile([128, RPC], F32, tag="sp", bufs=5,
                                      name=f"wp{jt}_{i}")
                    nc.tensor.matmul(wps[:], lhsT=_mm(wu[:, 0:128]), rhs=_mm(wu[:]),
                                     start=True, stop=True)
            mkt = stream.tile([128, RPC], BF16, tag="mask")
            nc.sync.dma_start(mkt[:], maskt[jt * 128:(jt + 1) * 128, :])
            cur_tm = []
            for k in range(K_HOPS):
                sp = psum_s.tile([128, RPC], F32, tag="sp", bufs=5)
                nc.tensor.matmul(
                    sp[:],
                    lhsT=_mm(qft_sb[32 * k:32 * k + 32, jt * 128:(jt + 1) * 128]),
                    rhs=_mm(yrt_sb[32 * k:32 * k + 32, :]),
                    start=True,
                    stop=True,
                )
                tt = tbuf.tile([128, RPC], BF16, tag="tt", bufs=8)
                nc.scalar.activation(tt[:], sp[:], ACTF.Exp)
                tm = tbuf.tile([128, RPC], BF16, tag="tm", bufs=16)
                nc.vector.tensor_mul(tm[:], tt[:], mkt[:])
                cur_tm.append(tm)
            pend.append((jt, cur_tm))
            # every 4 j-tiles, drain the pending u-matmuls as one dense
            # 12-matmul PE burst (long enough to cross the HAM warm window)
            if len(pend) == 4 and jt != KT - 1:
                drain_u(last=False)
        drain_u(last=True)

        # ---- B: vT = (u / rowsum), hop k at partitions 32k (one batched recip)
        vt_sb = small.tile([96, RPC], MDT)
        for k in range(K_HOPS):
            srow = small.tile([1, RPC], F32, tag=f"srow{k}", name=f"srow{k}")
            nc.vector.tensor_copy(srow[:], ups[k][32:33, :])
            rec = small.tile([1, RPC], F32, tag=f"rec{k}", name=f"rec{k}")
            nc.vector.reciprocal_approx_fast(rec[:], srow[:])
            bc = small.tile([32, RPC], F32, tag=f"bc{k}", name=f"bc{k}")
            nc.gpsimd.partition_broadcast(bc[:], rec[:])
            nc.vector.tensor_mul(vt_sb[32 * k:32 * k + 32, :], ups[k][0:32, :], bc[:])

        # ---- C2: g accumulation + epilogue per i-block
        acc2 = [psum_s.tile([128, D], F32, tag="sp", bufs=5, name=f"acc2_{m}")
                for m in range(MTI)]
        for m in range(MTI):
            for k in range(K_HOPS):
                nc.tensor.matmul(
                    acc2[m][:],
                    lhsT=_mm(vt_sb[32 * k:32 * k + 32, m * 128:(m + 1) * 128]),
                    rhs=_mm(ukts_sb[32 * k:32 * k + 32, :]),
                    start=(k == 0),
                    stop=(k == K_HOPS - 1),
                )
        sd_all = small.tile([128, MTI], F32)
        rstd_all = small.tile([128, MTI], F32)
        xcs = []
        for m in range(MTI):
            x = outp.tile([128, D], F32, tag="x")
            nc.vector.tensor_add(x[:], acc2[m][:], x0[m][:])          # H_half
            cl = outp.tile([128, D], F32, tag="cl")
            nc.vector.tensor_max(cl[:], x[:], nthr_sb[:])
            nc.vector.tensor_tensor(cl[:], cl[:], thr_sb[:], ALU.min)
            y2 = outp.tile([128, D], F32, tag="y2")
            nc.vector.tensor_sub(y2[:], x[:], cl[:])             # soft-threshold
            nc.vector.tensor_add(y2[:], y2[:], hr_sb[:, m, :])   # residual
            # layernorm stats
            red = outp.tile([128, 1], F32, tag="red")
            nc.vector.tensor_reduce(red[:], y2[:], AX.X, ALU.add)
            mu = outp.tile([128, 1], F32, tag="mu")
            nc.scalar.mul(mu[:], red[:], 1.0 / D)
            xc = outp.tile([128, D], F32, tag=f"xc{m}", name=f"xc{m}")
            nc.vector.tensor_scalar_sub(xc[:], y2[:], mu[:])
            var = outp.tile([128, 1], F32, tag="var")
            sq = outp.tile([128, D], F32, tag="sq")
            nc.vector.scalar_tensor_tensor(
                sq[:], in0=xc[:], scalar=1.0, in1=xc[:],
                op0=ALU.mult, op1=ALU.mult, accum_out=var[:],
            )
            nc.scalar.activation(sd_all[:, m:m + 1], var[:], ACTF.Sqrt,
                                 bias=eps_sb[:], scale=1.0 / D)
            xcs.append(xc)
        nc.vector.reciprocal_approx_fast(rstd_all[:], sd_all[:])
        for m in range(MTI):
            fin = outp.tile([128, D], F32, tag="fin")
            nc.vector.scalar_tensor_tensor(
                fin[:], in0=xcs[m][:], scalar=rstd_all[:, m:m + 1], in1=gam_sb[:],
                op0=ALU.mult, op1=ALU.mult,
            )
            nc.vector.tensor_add(fin[:], fin[:], bet_sb[:])
            nc.sync.dma_start(hout[m * 128:(m + 1) * 128, :], fin[:])
    nc.compile()
    return nc


# --------------------------------------------------------- NEFF-C: lap_smooth
def _build_lap():
    nc = bacc.Bacc("TRN2", target_bir_lowering=False, debug=False, num_devices=NC)
    lt = nc.dram_tensor("lt", [N, RPC], BF16, kind="ExternalInput").ap()
    xf = nc.dram_tensor("xf", [N, D], BF16, kind="ExternalInput").ap()
    xr = nc.dram_tensor("xr", [RPC, D], F32, kind="ExternalInput").ap()
    part = nc.dram_tensor("part", [128, MTI], F32, kind="ExternalOutput").ap()

    with tile.TileContext(nc) as tc, ExitStack() as ctx:
        res = ctx.enter_context(tc.tile_pool(name="res", bufs=1))
        stream = ctx.enter_context(tc.tile_pool(name="stream", bufs=4))
        outp = ctx.enter_context(tc.tile_pool(name="outp", bufs=4))
        psum = ctx.enter_context(tc.tile_pool(name="psum", bufs=1, space="PSUM"))

        xfsb = res.tile([128, KT, D], BF16)
        xf_re = xf.rearrange("(t p) c -> p t c", p=128)
        nc.sync.dma_start(xfsb[:, 0:8, :], xf_re[:, 0:8, :])
        xrsb = res.tile([128, MTI, D], F32)
        nc.sync.dma_start(xrsb[:], xr.rearrange("(t p) c -> p t c", p=128))
        pcol = res.tile([128, MTI], F32)

        ps = [psum.tile([128, D], F32, tag=f"ps{m}", name=f"ps{m}") for m in range(MTI)]
        ltts = {}
        for k in range(KT):
            ltt = stream.tile([128, RPC], BF16, tag="ltt", bufs=10, name=f"ltt{k}")
            nc.sync.dma_start(ltt[:], lt[k * 128:(k + 1) * 128, :])
            ltts[k] = ltt
            if k % 8 == 0 and k < 24:
                c = k // 8 + 1
                nc.sync.dma_start(xfsb[:, 8 * c:8 * (c + 1), :], xf_re[:, 8 * c:8 * (c + 1), :])
            if k % 8 == 7:
                for kk in range(k - 7, k + 1):
                    for m in range(MTI):
                        nc.tensor.matmul(
                            ps[m][:],
                            lhsT=_mm(ltts[kk][:, m * 128:(m + 1) * 128]),
                            rhs=_mm(xfsb[:, kk, :]),
                            start=(kk == 0),
                            stop=(kk == KT - 1),
                        )
        for m in range(MTI):
            lhs = outp.tile([128, D], F32, tag="lhs")
            nc.vector.tensor_copy(lhs[:], ps[m][:])
            prod = outp.tile([128, D], F32, tag="prod")
            nc.vector.scalar_tensor_tensor(
                prod[:], in0=lhs[:], scalar=1.0, in1=xrsb[:, m, :],
                op0=ALU.mult, op1=ALU.mult, accum_out=pcol[:, m:m + 1],
            )
        nc.sync.dma_start(part[:], pcol[:])
    nc.compile()
    return nc


def _get(name):
    if name not in _cache:
        _cache[name] = {"hop": _build_hop, "main": _build_main, "lap": _build_lap}[name]()
    return _cache[name]


# ------------------------------------------------------------------- driver
def kernel(H, A, adj_mask, L, U, lambda_laps, hop_weights, threshold, gamma, beta):
    H = np.asarray(H, np.float32)
    A = np.asarray(A, np.float32)
    adj_mask = np.asarray(adj_mask, np.float32)
    L = np.asarray(L, np.float32)
    U = np.asarray(U, np.float32)
    lambda_laps = np.asarray(lambda_laps, np.float32)
    hop_weights = np.asarray(hop_weights, np.float32)
    threshold = np.asarray(threshold, np.float32)
    gamma = np.asarray(gamma, np.float32)
    beta = np.asarray(beta, np.float32)

    LAST_EXEC_NS.clear()

    AT = np.ascontiguousarray(A.T)
    LTf = np.ascontiguousarray(L.T)
    MKT = np.ascontiguousarray(adj_mask.T)

    def rows(c):
        return slice(c * RPC, (c + 1) * RPC)

    # ---- L1/L2: hop matmuls
    hop_nc = _get("hop")
    at_c = [np.ascontiguousarray(AT[:, rows(c)]).astype(ml_dtypes.bfloat16)
            for c in range(NC)]
    r1 = _run(hop_nc, [{"at": at_c[c], "x": H.astype(ml_dtypes.bfloat16)}
                       for c in range(NC)], "L1_hop1")
    H1 = np.concatenate([r1[c]["y"] for c in range(NC)], axis=0)
    h1b = H1.astype(ml_dtypes.bfloat16)
    r2 = _run(hop_nc, [{"at": at_c[c], "x": h1b} for c in range(NC)], "L2_hop2")
    H2 = np.concatenate([r2[c]["y"] for c in range(NC)], axis=0)

    # ---- host: small algebra
    w = np.exp(hop_weights - hop_weights.max())
    w = (w / w.sum()).astype(np.float32)
    laps = np.log1p(np.exp(lambda_laps.astype(np.float64))).astype(np.float32)
    coeff = np.float64(R / (N * EPS_SUB ** 2))
    hops = [H, H1, H2]

    qft3 = np.empty((96, N), np.float32)
    yft3 = np.empty((96, N), np.float32)
    qf1s = []
    ukts3 = np.empty((96, D), np.float32)
    ssc = np.zeros((N, D), np.float32)
    for k in range(K_HOPS):
        Yf = hops[k] @ U[k]                       # [N, R]
        G = Yf.T @ Yf
        M = np.eye(R, dtype=np.float64) + coeff * G.astype(np.float64)
        Minv = np.linalg.inv(M).astype(np.float32)
        Qf = Yf @ Minv
        qft3[32 * k:32 * k + 32, :] = Qf.T
        yft3[32 * k:32 * k + 32, :] = Yf.T
        q1 = np.zeros((N, 34), np.float32)
        q1[:, :R] = Qf
        q1[:, R] = 1.0
        qf1s.append(q1)
        ukts3[32 * k:32 * k + 32, :] = (ETA * w[k]) * U[k].T
        ssc -= (ETA * laps[k]) * hops[k]
    thrb = np.broadcast_to(threshold, (128, D)).astype(np.float32).copy()
    gamb = np.broadcast_to(gamma, (128, D)).astype(np.float32).copy()
    betb = np.broadcast_to(beta, (128, D)).astype(np.float32).copy()

    # ---- L3: attention + output rows
    main_nc = _get("main")
    lt_c = [np.ascontiguousarray(LTf[:, rows(c)]).astype(ml_dtypes.bfloat16)
            for c in range(NC)]
    in3 = []
    for c in range(NC):
        m = {
            "maskt": np.ascontiguousarray(MKT[:, rows(c)]).astype(ml_dtypes.bfloat16),
            "lt": lt_c[c],
            "ssc": ssc.astype(ml_dtypes.bfloat16),
            "hr": np.ascontiguousarray(H[rows(c)]),
            "qft3": qft3.astype(ml_dtypes.bfloat16),
            "yrt3": np.ascontiguousarray(yft3[:, rows(c)]).astype(ml_dtypes.bfloat16),
            "ukts3": ukts3,
            "thrb": thrb,
            "nthrb": -thrb,
            "gamb": gamb,
            "betb": betb,
        }
        for k in range(K_HOPS):
            m[f"qf1_{k}"] = qf1s[k].astype(ml_dtypes.bfloat16)
        in3.append(m)
    r3 = _run(main_nc, in3, "L3_main")
    H_out = np.concatenate([r3[c]["hout"] for c in range(NC)], axis=0)

    # ---- L4: lap_smooth partials
    lap_nc = _get("lap")
    hob = H_out.astype(ml_dtypes.bfloat16)
    in4 = [{"lt": np.ascontiguousarray(LTf[:, rows(c)]).astype(ml_dtypes.bfloat16),
            "xf": hob, "xr": np.ascontiguousarray(H_out[rows(c)])}
           for c in range(NC)]
    r4 = _run(lap_nc, in4, "L4_lap")
    lap_smooth = np.float32(sum(float(r4[c]["part"].sum()) for c in range(NC)))

    # ---- host: orth loss (depends only on U input)
    orth = np.float32(0.0)
    I_r = np.eye(R, dtype=np.float32)
    for k in range(K_HOPS):
        for l in range(k, K_HOPS):
            Gkl = U[k].T @ U[l]
            Gkl = Gkl - I_r if l == k else Gkl
            orth = orth + np.float32((Gkl ** 2).sum())

    return H_out, orth, lap_smooth, w
